# revision 1
# baseline (speedup 1.0000x reference)
"""Trainium2 Bass kernel for nn_ModelMultitaskBinary (MMoE multitask binary loss).

Strategy: data-parallel over batch B=512 across 8 cores (64 samples -> 1920
candidate rows per core). All params replicated. No collectives: each core
emits its 64 per-sample losses; the host averages 512 values.

On-chip pipeline per core (activations feature-major [feat(part), row(free)],
matmul inputs bf16, accumulation fp32 in PSUM):
  xT -> h1 = relu(fc1) -> h = fc2 -> glog (row-major via lhsT=h slices)
  -> top-3-of-6 gating (DVE, batched) -> experts in row-group blocks:
  ehT (feature-major), eo (row-major, staged to SBUF bf16),
  y_t += gate*eo on DVE (bf16 perf modes) -> per task: PE-transpose y,
  tower1, logits (row-major) -> BCE + aux load-balance loss -> [64] losses.

Two program variants: zero_bias (all bias inputs are zeros, per the spec
fills: skips bias adds / bias matmuls) and the general fallback.
"""
import os
import sys
from contextlib import ExitStack

for _p in ("/opt/trn_rl_repo", "/root/.axon_site/_ro/trn_rl_repo"):
    if os.path.isdir(_p) and _p not in sys.path:
        sys.path.insert(0, _p)

import numpy as np
import ml_dtypes

import concourse.bass as bass
import concourse.tile as tile
from concourse import bacc, mybir
from concourse.masks import make_identity
from concourse.bass_utils import run_bass_kernel_spmd

F32 = mybir.dt.float32
BF16 = mybir.dt.bfloat16
BF = ml_dtypes.bfloat16
AF = mybir.ActivationFunctionType
OP = mybir.AluOpType
AX = mybir.AxisListType

NCORES = 8
B, C, T, H, E, EH, TH = 512, 30, 3, 512, 6, 512, 512
BSH = B // NCORES          # 64 samples per core
R = BSH * C                # 1920 rows per core
NRT = R // 128             # 15 row tiles
KC = H // 128              # 4 feature chunks
RS = [(0, 512), (512, 1024), (1024, 1536), (1536, R)]  # row slices (<=512)
RG_RT = [(0, 4), (4, 8), (8, 12), (12, 15)]            # row tiles per group
LOSS_COEF = 0.01

# engine-assignment knobs (tuned against the timeline cost model)
KNOBS = {
    "eo_copy_dve_every": 2,   # every k-th eo psum->sbuf copy goes to DVE
    "ts_pool_every": 4,       # every k-th gate-scale mult on GpSimd
    "add_pool_every": 5,      # every k-th y-accumulate add goes to GpSimd
    "ytr_copy_act_every": 2,  # every k-th y-transpose psum->sbuf copy on ACT
}

_CACHED = {}


def build_nc(zero_bias: bool):
    nc = bacc.Bacc(None, target_bir_lowering=False, debug=False)

    xT_d = nc.dram_tensor("xT", [KC, 128, R], BF16, kind="ExternalInput")
    scores_d = nc.dram_tensor("scores", [BSH, T, C], F32, kind="ExternalInput")
    fc1w_d = nc.dram_tensor("fc1w", [KC, 128, H], BF16, kind="ExternalInput")
    fc1b_d = nc.dram_tensor("fc1b", [128, KC], F32, kind="ExternalInput")
    fc2w_d = nc.dram_tensor("fc2w", [KC, 128, H], BF16, kind="ExternalInput")
    fc2b_d = nc.dram_tensor("fc2b", [128, KC], F32, kind="ExternalInput")
    wg_d = nc.dram_tensor("wg", [KC, 128, T * E], BF16, kind="ExternalInput")
    ew1_d = nc.dram_tensor("ew1", [E, KC, 128, EH], BF16, kind="ExternalInput")
    eb1_d = nc.dram_tensor("eb1", [E, 128, KC], F32, kind="ExternalInput")
    ew2_d = nc.dram_tensor("ew2", [E, KC, 128, H], BF16, kind="ExternalInput")
    eb2_d = nc.dram_tensor("eb2", [E, 1, H], BF16, kind="ExternalInput")
    tw1_d = nc.dram_tensor("tw1", [T, KC, 128, TH], BF16, kind="ExternalInput")
    tb1_d = nc.dram_tensor("tb1", [T, 128, KC], F32, kind="ExternalInput")
    tw2_d = nc.dram_tensor("tw2", [T, 128, KC], BF16, kind="ExternalInput")
    tb2_d = nc.dram_tensor("tb2", [128, T], F32, kind="ExternalInput")
    sel_d = nc.dram_tensor("sel", [NRT, 128, BSH], F32, kind="ExternalInput")
    selt_d = nc.dram_tensor("selt", [NRT, BSH, 128], F32, kind="ExternalInput")
    srm_d = nc.dram_tensor("srm", [128, NRT, T], F32, kind="ExternalInput")
    loss_d = nc.dram_tensor("loss", [BSH, 1], F32, kind="ExternalOutput")

    eo_dve = KNOBS["eo_copy_dve_every"]
    ts_pool = KNOBS.get("ts_pool_every", 5)
    add_pool = KNOBS["add_pool_every"]
    ytr_act = KNOBS["ytr_copy_act_every"]

    with tile.TileContext(nc, pool_alloc_mode="queue") as tc, ExitStack() as ctx:
        perm = ctx.enter_context(tc.tile_pool(name="perm", bufs=1))
        dram = ctx.enter_context(tc.tile_pool(name="dram", bufs=1, space="DRAM"))
        psA = ctx.enter_context(tc.tile_pool(name="psA", bufs=5, space="PSUM"))
        psB = ctx.enter_context(tc.tile_pool(name="psB", bufs=2, space="PSUM"))
        hpool = ctx.enter_context(tc.tile_pool(name="hpool", bufs=1))

        ident_bf = perm.tile([128, 128], BF16)
        make_identity(nc, ident_bf)
        ident_f = perm.tile([128, 128], F32)
        make_identity(nc, ident_f)
        if not zero_bias:
            ones_bf = perm.tile([1, 128], BF16)
            nc.vector.memset(ones_bf, 1.0)

        warm = perm.tile([128, 1], F32)
        nc.scalar.activation(warm, ident_f[:, 0:1], AF.Exp)
        nc.scalar.activation(warm, ident_f[:, 0:1], AF.Abs)
        nc.scalar.activation(warm, ident_f[:, 0:1], AF.Ln, bias=1.0)

        scores_sb = perm.tile([BSH, T, C], F32)
        nc.sync.dma_start(scores_sb, scores_d[:, :, :])
        srm_sb = perm.tile([128, NRT, T], F32)
        sel_sb = [perm.tile([128, BSH], F32, name=f"sel{rt}") for rt in range(NRT)]
        selt_sb = [perm.tile([BSH, 128], F32, name=f"selt{rt}") for rt in range(NRT)]
        if not zero_bias:
            tb2_sb = perm.tile([128, 1, T], F32)
            nc.sync.dma_start(tb2_sb, tb2_d[:, :])

        glog = perm.tile([128, NRT * T * E], F32)    # [128, 270] row-major
        gates = perm.tile([128, NRT * T * E], F32)
        gates_fm = perm.tile([T * E, R], F32)        # [18, 1920] feature-major
        ypool = ctx.enter_context(tc.tile_pool(name="ypool", bufs=1))
        yT = [ypool.tile([128, KC * R], BF16, name=f"yT{t}") for t in range(T)]
        logits_sb = perm.tile([128, NRT, T], F32)

        hT = [hpool.tile([128, R], BF16, name=f"hT{k}") for k in range(KC)]

        # expert weights: resident for the whole expert phase
        epool = ctx.enter_context(tc.tile_pool(name="epool", bufs=1))

        # ---------------- phase 1+2: shared bottom ----------------
        with tc.tile_pool(name="early", bufs=1) as early:
            fc1w = [early.tile([128, H], BF16, name=f"fc1w{k}")
                    for k in range(KC)]
            fc2w = [early.tile([128, H], BF16, name=f"fc2w{k}")
                    for k in range(KC)]
            wgw = [early.tile([128, T * E], BF16, name=f"wg{k}")
                   for k in range(KC)]
            xT = [early.tile([128, R], BF16, name=f"xT{k}") for k in range(KC)]
            r0, r1 = RS[0]
            for k in range(KC):
                nc.sync.dma_start(fc1w[k], fc1w_d[k, :, :])
                nc.sync.dma_start(xT[k][:, r0:r1], xT_d[k, :, r0:r1])
            for k in range(KC):
                nc.sync.dma_start(wgw[k], wg_d[k, :, :])
                nc.sync.dma_start(fc2w[k], fc2w_d[k, :, :])
            for (r0, r1) in RS[1:]:
                for k in range(KC):
                    nc.sync.dma_start(xT[k][:, r0:r1], xT_d[k, :, r0:r1])
            if not zero_bias:
                fc1b = early.tile([128, KC], F32)
                nc.sync.dma_start(fc1b, fc1b_d[:, :])
                fc2b = early.tile([128, KC], F32)
                nc.sync.dma_start(fc2b, fc2b_d[:, :])

            ew1 = [[None] * KC for _ in range(E)]
            ew2 = [[None] * KC for _ in range(E)]
            eb1 = [None] * E
            eb2row = [None] * E
            for e in range(E):
                for k in range(KC):
                    w1 = epool.tile([128, EH], BF16, name=f"ew1_{e}_{k}")
                    nc.sync.dma_start(w1, ew1_d[e, k, :, :])
                    ew1[e][k] = w1
                    w2 = epool.tile([128, H], BF16, name=f"ew2_{e}_{k}")
                    nc.sync.dma_start(w2, ew2_d[e, k, :, :])
                    ew2[e][k] = w2
                if not zero_bias:
                    b1 = epool.tile([128, KC], F32, name=f"eb1_{e}")
                    nc.sync.dma_start(b1, eb1_d[e, :, :])
                    eb1[e] = b1
                    b2r = epool.tile([1, H], BF16, name=f"eb2_{e}")
                    nc.sync.dma_start(b2r, eb2_d[e, :, :])
                    eb2row[e] = b2r

            nc.sync.dma_start(srm_sb, srm_d[:, :, :])
            for rt in range(NRT):
                nc.sync.dma_start(sel_sb[rt], sel_d[rt, :, :])
                nc.sync.dma_start(selt_sb[rt], selt_d[rt, :, :])

            h1T = [early.tile([128, R], BF16, name=f"h1T{k}") for k in range(KC)]
            for mc in range(KC):
                for (r0, r1) in RS:
                    ps = psA.tile([128, r1 - r0], F32, name="accB", tag="acc")
                    for k in range(KC):
                        nc.tensor.matmul(
                            ps, fc1w[k][:, mc * 128:(mc + 1) * 128], xT[k][:, r0:r1],
                            start=(k == 0), stop=(k == KC - 1))
                    if zero_bias:
                        nc.scalar.activation(h1T[mc][:, r0:r1], ps, AF.Relu)
                    else:
                        nc.scalar.activation(h1T[mc][:, r0:r1], ps, AF.Relu,
                                             bias=fc1b[:, mc:mc + 1])
            for mc in range(KC):
                for (r0, r1) in RS:
                    ps = psA.tile([128, r1 - r0], F32, name="accB2", tag="acc")
                    for k in range(KC):
                        nc.tensor.matmul(
                            ps, fc2w[k][:, mc * 128:(mc + 1) * 128], h1T[k][:, r0:r1],
                            start=(k == 0), stop=(k == KC - 1))
                    if zero_bias:
                        nc.scalar.activation(hT[mc][:, r0:r1], ps, AF.Copy)
                    else:
                        nc.scalar.activation(hT[mc][:, r0:r1], ps, AF.Identity,
                                             bias=fc2b[:, mc:mc + 1])

            # ---------------- phase 3: gate logits (row-major) ----------------
            GE = T * E
            for rt in range(NRT):
                ps = psA.tile([128, GE], F32, name="accG", tag="acc")
                for k in range(KC):
                    nc.tensor.matmul(
                        ps, hT[k][:, rt * 128:(rt + 1) * 128], wgw[k],
                        start=(k == 0), stop=(k == KC - 1))
                nc.scalar.activation(glog[:, rt * GE:(rt + 1) * GE], ps, AF.Copy)

        # ---------------- gating: top-3-of-6 masked softmax ----------------
        NG = NRT * T  # 45 groups of E
        v = glog.rearrange("p (g e) -> p g e", e=E)
        gtmp = ctx.enter_context(tc.tile_pool(name="gtmp", bufs=1))  # noqa
        neginf = gtmp.tile([128, NG, E], F32)
        nc.vector.memset(neginf, -1e30)
        m1 = gtmp.tile([128, NG, 1], F32)
        nc.vector.tensor_reduce(m1, v, AX.X, OP.max)
        m1b = m1.broadcast_to([128, NG, E])
        mask = gtmp.tile([128, NG, E], mybir.dt.uint8)
        nc.vector.tensor_tensor(mask, v, m1b, OP.is_ge)
        v2 = gtmp.tile([128, NG, E], F32)
        nc.vector.select(v2, mask, neginf, v)
        m2 = gtmp.tile([128, NG, 1], F32)
        nc.vector.tensor_reduce(m2, v2, AX.X, OP.max)
        mask2 = gtmp.tile([128, NG, E], mybir.dt.uint8)
        nc.vector.tensor_tensor(mask2, v2, m2.broadcast_to([128, NG, E]), OP.is_ge)
        v3 = gtmp.tile([128, NG, E], F32)
        nc.vector.select(v3, mask2, neginf, v2)
        m3 = gtmp.tile([128, NG, 1], F32)
        nc.vector.tensor_reduce(m3, v3, AX.X, OP.max)
        keep = gtmp.tile([128, NG, E], F32)
        nc.vector.tensor_tensor(keep, v, m3.broadcast_to([128, NG, E]), OP.is_ge)
        vs = gtmp.tile([128, NG, E], F32)
        nc.vector.tensor_tensor(vs, v, m1b, OP.subtract)
        ex = gtmp.tile([128, NG, E], F32)
        nc.scalar.activation(ex, vs, AF.Exp)
        ek = gtmp.tile([128, NG, E], F32)
        nc.vector.tensor_tensor(ek, ex, keep, OP.mult)
        ssum = gtmp.tile([128, NG, 1], F32)
        nc.vector.tensor_reduce(ssum, ek, AX.X, OP.add)
        rsum = gtmp.tile([128, NG, 1], F32)
        nc.vector.reciprocal(rsum, ssum)
        gv = gates.rearrange("p (g e) -> p g e", e=E)
        nc.vector.tensor_tensor(gv, ek, rsum.broadcast_to([128, NG, E]), OP.mult)

        # gates feature-major (for aux loss): PE transpose per row tile
        GE = T * E
        for rt in range(NRT):
            gp = psB.tile([GE, 128], F32, name="gtr", tag="small", bufs=1)
            nc.tensor.transpose(gp, gates[:, rt * GE:(rt + 1) * GE], ident_f)
            nc.vector.tensor_copy(gates_fm[:, rt * 128:(rt + 1) * 128], gp)

        # aux: imp[t,e,b] = sum_c gates_fm -> cv^2 per (b,t)
        imp = perm.tile([T * E, BSH], F32)
        nc.vector.tensor_reduce(
            imp, gates_fm.rearrange("p (b c) -> p b c", c=C), AX.X, OP.add)
        ip = psB.tile([BSH, T * E], F32, name="itr", tag="small", bufs=1)
        nc.tensor.transpose(ip, imp, ident_f[:T * E, :T * E])
        impT = perm.tile([BSH, T * E], F32)
        nc.vector.tensor_copy(impT, ip)
        impTv = impT.rearrange("b (t e) -> b t e", e=E)
        auxs = perm.tile([BSH, 1], F32)
        for t in range(T):
            st = perm.tile([BSH, 6], F32, name=f"bnst{t}")
            nc.vector.bn_stats(st, impTv[:, t, :])
            mv = perm.tile([BSH, 2], F32, name=f"bnmv{t}")
            nc.vector.bn_aggr(mv, st)
            msq = perm.tile([BSH, 1], F32, name=f"msq{t}")
            nc.vector.tensor_tensor(msq, mv[:, 0:1], mv[:, 0:1], OP.mult)
            nc.vector.tensor_scalar(msq, msq, 1e-10, None, OP.add)
            rec = perm.tile([BSH, 1], F32, name=f"rec{t}")
            nc.vector.reciprocal(rec, msq)
            cv2 = perm.tile([BSH, 1], F32, name=f"cv2{t}")
            nc.vector.tensor_tensor(cv2, mv[:, 1:2], rec, OP.mult)
            if t == 0:
                nc.vector.tensor_copy(auxs, cv2)
            else:
                nc.vector.tensor_tensor(auxs, auxs, cv2, OP.add)

        # ------------- phase 4: experts, row-group blocked -------------
        nco = 0  # rotating index for engine-split knobs
        _st = {"n": 0}

        def emit_transposes(rg):
            pt0, pt1, pyg = rg
            for rtl in range(pt1 - pt0):
                rt = pt0 + rtl
                for t in range(T):
                    tp = psB.tile([128, KC, 128], BF16, name="ytr", tag="tr",
                                  bufs=2)
                    for jc in range(KC):
                        nc.tensor.transpose(
                            tp[:, jc, :], pyg[t][rtl][:, jc * 128:(jc + 1) * 128],
                            ident_bf)
                    dst = bass.AP(
                        tensor=yT[t].tensor, offset=yT[t].offset + rt * 128,
                        ap=[yT[t].ap[0], [R, KC], [1, 128]])
                    _st["n"] += 1
                    if _st["n"] % ytr_act == 0:
                        nc.scalar.activation(dst, tp, AF.Copy)
                    else:
                        nc.vector.tensor_copy(dst, tp)

        prev_rg = None
        with tc.tile_pool(name="exp", bufs=2) as exp:
            for gi, ((r0, r1), (t0, t1)) in enumerate(zip(RS, RG_RT)):
                rgw = r1 - r0
                yg = [[exp.tile([128, H], BF16, name=f"yg{t}_{rtl}", tag="yg",
                                bufs=20) for rtl in range(t1 - t0)]
                      for t in range(T)]
                for e in range(E):
                    if e == 2 and prev_rg is not None:
                        emit_transposes(prev_rg)
                        prev_rg = None
                    ehs = [exp.tile([128, rgw], BF16, name=f"ehs{k}",
                                    tag=f"ehs{k}") for k in range(KC)]
                    for mc in range(KC):
                        ps = psA.tile([128, rgw], F32, name="accE", tag="acc")
                        for k in range(KC):
                            nc.tensor.matmul(
                                ps, ew1[e][k][:, mc * 128:(mc + 1) * 128],
                                hT[k][:, r0:r1],
                                start=(k == 0), stop=(k == KC - 1))
                        if zero_bias:
                            nc.scalar.activation(ehs[mc], ps, AF.Relu)
                        else:
                            nc.scalar.activation(ehs[mc], ps, AF.Relu,
                                                 bias=eb1[e][:, mc:mc + 1])
                    for rtl in range(t1 - t0):
                        rt = t0 + rtl
                        ps = psA.tile([128, H], F32, name="accO", tag="acc")
                        for k in range(KC):
                            nc.tensor.matmul(
                                ps, ehs[k][:, rtl * 128:(rtl + 1) * 128], ew2[e][k],
                                start=(k == 0),
                                stop=(k == KC - 1) and zero_bias)
                        if not zero_bias:
                            nc.tensor.matmul(ps, ones_bf, eb2row[e],
                                             start=False, stop=True)
                        # evacuate eo once; combine from SBUF bf16 (fast modes)
                        eo = exp.tile([128, H], BF16, name="eo", tag="eo",
                                      bufs=6)
                        nco += 1
                        if (nco % eo_dve == 0) if eo_dve > 0 else (nco % -eo_dve != 0):
                            nc.vector.tensor_copy(eo, ps)
                        else:
                            nc.scalar.activation(eo, ps, AF.Copy)
                        for t in range(T):
                            g_ap = gates[:, rt * 18 + t * 6 + e:
                                         rt * 18 + t * 6 + e + 1]
                            nco += 1
                            if e == 0:
                                nc.vector.tensor_scalar(
                                    yg[t][rtl], eo, g_ap, None, OP.mult)
                            else:
                                tmp = exp.tile([128, H], BF16, name="ysc",
                                               tag="ysc", bufs=4)
                                if nco % ts_pool == 0:
                                    nc.gpsimd.tensor_scalar(tmp, eo, g_ap,
                                                            None, OP.mult)
                                else:
                                    nc.vector.tensor_scalar(tmp, eo, g_ap,
                                                            None, OP.mult)
                                if nco % add_pool == 0:
                                    nc.gpsimd.tensor_tensor(
                                        yg[t][rtl], yg[t][rtl], tmp, OP.add)
                                else:
                                    nc.vector.tensor_tensor(
                                        yg[t][rtl], yg[t][rtl], tmp, OP.add)
                prev_rg = (t0, t1, yg)
            emit_transposes(prev_rg)

        # labels in row-major layout: smax -> broadcast (selector matmuls)
        smax = perm.tile([BSH, T], F32)
        smax3 = perm.tile([BSH, T, 1], F32)
        nc.vector.tensor_reduce(smax3, scores_sb, AX.X, OP.max)
        nc.vector.tensor_copy(smax, smax3.rearrange("b t one -> b (t one)"))
        smax_bc = perm.tile([128, NRT, T], F32)
        for rt in range(NRT):
            pb = psB.tile([128, T], F32, name="smb", tag="small", bufs=1)
            nc.tensor.matmul(pb, selt_sb[rt], smax, start=True, stop=True)
            nc.vector.tensor_copy(smax_bc[:, rt, :], pb)
        labels_rm = perm.tile([128, NRT, T], F32)
        nc.vector.tensor_tensor(labels_rm, srm_sb, smax_bc, OP.is_equal)

        # ---------------- phase 5: towers ----------------
        with tc.tile_pool(name="tow", bufs=2) as tow:
            for t in range(T):
                tw1 = []
                for k in range(KC):
                    w1 = tow.tile([128, TH], BF16, name=f"tw1_{k}", tag=f"tw1_{k}")
                    nc.sync.dma_start(w1, tw1_d[t, k, :, :])
                    tw1.append(w1)
                if not zero_bias:
                    tb1 = tow.tile([128, KC], F32, tag="tb1")
                    nc.sync.dma_start(tb1, tb1_d[t, :, :])
                tw2 = tow.tile([128, KC], BF16, tag="tw2")
                nc.sync.dma_start(tw2, tw2_d[t, :, :])

                thT = [tow.tile([128, R], BF16, name=f"thT{k}", tag=f"thT{k}", bufs=1)
                       for k in range(KC)]
                for mc in range(KC):
                    for (r0, r1) in RS:
                        ps = psA.tile([128, r1 - r0], F32, name="accT", tag="acc")
                        for k in range(KC):
                            nc.tensor.matmul(
                                ps, tw1[k][:, mc * 128:(mc + 1) * 128], yT[t][:, k * R + r0:k * R + r1],
                                start=(k == 0), stop=(k == KC - 1))
                        if zero_bias:
                            nc.scalar.activation(thT[mc][:, r0:r1], ps, AF.Relu)
                        else:
                            nc.scalar.activation(thT[mc][:, r0:r1], ps, AF.Relu,
                                                 bias=tb1[:, mc:mc + 1])
                for rt in range(NRT):
                    pl = psB.tile([128, 1], F32, name="lg", tag="small", bufs=1)
                    for k in range(KC):
                        nc.tensor.matmul(
                            pl, thT[k][:, rt * 128:(rt + 1) * 128], tw2[:, k:k + 1],
                            start=(k == 0), stop=(k == KC - 1))
                    nc.vector.tensor_copy(logits_sb[:, rt, t:t + 1], pl)

        # ---------------- phase 6: BCE (row-major) ----------------
        lg = logits_sb  # [128, NRT, T]
        if not zero_bias:
            nc.vector.tensor_tensor(lg, lg, tb2_sb.broadcast_to([128, NRT, T]),
                                    OP.add)
        t1_ = perm.tile([128, NRT, T], F32)
        nc.vector.tensor_scalar(t1_, lg, 0.0, None, OP.max)
        t2_ = perm.tile([128, NRT, T], F32)
        nc.vector.tensor_tensor(t2_, lg, labels_rm, OP.mult)
        absl = perm.tile([128, NRT, T], F32)
        nc.scalar.activation(absl, lg, AF.Abs)
        expl = perm.tile([128, NRT, T], F32)
        nc.scalar.activation(expl, absl, AF.Exp, scale=-1.0)
        lp = perm.tile([128, NRT, T], F32)
        nc.scalar.activation(lp, expl, AF.Ln, bias=1.0)
        nc.vector.tensor_tensor(t1_, t1_, t2_, OP.subtract)
        nc.vector.tensor_tensor(t1_, t1_, lp, OP.add)
        bs = perm.tile([128, NRT], F32)
        nc.vector.tensor_reduce(bs, t1_, AX.X, OP.add)
        pb = psB.tile([BSH, 1], F32, name="bsum", tag="small", bufs=1)
        for rt in range(NRT):
            nc.tensor.matmul(pb, sel_sb[rt], bs[:, rt:rt + 1],
                             start=(rt == 0), stop=(rt == NRT - 1))
        tsum = perm.tile([BSH, 1], F32)
        nc.vector.tensor_copy(tsum, pb)

        loss_sb = perm.tile([BSH, 1], F32)
        nc.vector.tensor_scalar(loss_sb, tsum, 1.0 / (T * C), None, OP.mult)
        auxf = perm.tile([BSH, 1], F32)
        nc.vector.tensor_scalar(auxf, auxs, LOSS_COEF, None, OP.mult)
        nc.vector.tensor_tensor(loss_sb, loss_sb, auxf, OP.add)
        nc.sync.dma_start(loss_d[:, :], loss_sb)

    nc.compile()
    return nc


def get_nc(zero_bias=True):
    key = (zero_bias, tuple(sorted(KNOBS.items())))
    if key not in _CACHED:
        _CACHED[key] = build_nc(zero_bias)
    return _CACHED[key]



_SEL_CACHE = None


def _sel_mats():
    """0/1 selector matrices mapping rows r=rt*128+p to samples b=r//30."""
    global _SEL_CACHE
    if _SEL_CACHE is None:
        sel = np.zeros((NRT, 128, BSH), np.float32)
        for rt in range(NRT):
            for p in range(128):
                b = (rt * 128 + p) // C
                sel[rt, p, b] = 1.0
        selt = np.ascontiguousarray(sel.transpose(0, 2, 1))
        _SEL_CACHE = (sel, selt)
    return _SEL_CACHE


def host_prep(inputs):
    """Shard + cast + rearrange the full inputs into 8 per-core in_maps."""
    x = np.asarray(inputs["candidate_cls_embed"], np.float32)
    scores = np.asarray(inputs["scores"], np.float32)
    fc1_w = np.asarray(inputs["fc1_w"], np.float32)
    fc1_b = np.asarray(inputs["fc1_b"], np.float32)
    fc2_w = np.asarray(inputs["fc2_w"], np.float32)
    fc2_b = np.asarray(inputs["fc2_b"], np.float32)
    w_gate = np.asarray(inputs["w_gate"], np.float32)
    expert_w1 = np.asarray(inputs["expert_w1"], np.float32)
    expert_b1 = np.asarray(inputs["expert_b1"], np.float32)
    expert_w2 = np.asarray(inputs["expert_w2"], np.float32)
    expert_b2 = np.asarray(inputs["expert_b2"], np.float32)
    tower_w1 = np.asarray(inputs["tower_w1"], np.float32)
    tower_b1 = np.asarray(inputs["tower_b1"], np.float32)
    tower_w2 = np.asarray(inputs["tower_w2"], np.float32)
    tower_b2 = np.asarray(inputs["tower_b2"], np.float32)

    zero_bias = not (fc1_b.any() or fc2_b.any() or expert_b1.any()
                     or expert_b2.any() or tower_b1.any() or tower_b2.any())

    shared = {
        "fc1w": fc1_w.astype(BF).reshape(KC, 128, H),
        "fc1b": np.ascontiguousarray(fc1_b.reshape(KC, 128).T),
        "fc2w": fc2_w.astype(BF).reshape(KC, 128, H),
        "fc2b": np.ascontiguousarray(fc2_b.reshape(KC, 128).T),
        "wg": np.ascontiguousarray(w_gate.transpose(1, 0, 2).reshape(H, T * E))
              .astype(BF).reshape(KC, 128, T * E),
        "ew1": expert_w1.astype(BF).reshape(E, KC, 128, EH),
        "eb1": np.ascontiguousarray(
            expert_b1.reshape(E, KC, 128).transpose(0, 2, 1)),
        "ew2": expert_w2.astype(BF).reshape(E, KC, 128, H),
        "eb2": expert_b2.astype(BF).reshape(E, 1, H),
        "tw1": tower_w1.astype(BF).reshape(T, KC, 128, TH),
        "tb1": np.ascontiguousarray(
            tower_b1.reshape(T, KC, 128).transpose(0, 2, 1)),
        "tw2": np.ascontiguousarray(
            tower_w2.reshape(T, KC, 128).transpose(0, 2, 1)).astype(BF),
        "tb2": np.ascontiguousarray(
            np.broadcast_to(tower_b2[None, :], (128, T))),
        "sel": _sel_mats()[0],
        "selt": _sel_mats()[1],
    }
    in_maps = []
    for ci in range(NCORES):
        xs = x[ci * BSH:(ci + 1) * BSH].reshape(R, H)
        xT = np.ascontiguousarray(xs.T).astype(BF).reshape(KC, 128, R)
        m = dict(shared)
        m["xT"] = xT
        sc = np.ascontiguousarray(scores[ci * BSH:(ci + 1) * BSH])
        m["scores"] = sc
        srm = sc.transpose(0, 2, 1).reshape(NRT, 128, T).transpose(1, 0, 2)
        m["srm"] = np.ascontiguousarray(srm)
        in_maps.append(m)
    return in_maps, zero_bias


def kernel(**inputs) -> np.ndarray:
    in_maps, zero_bias = host_prep(inputs)
    nc = get_nc(zero_bias)
    res = run_bass_kernel_spmd(nc, in_maps, list(range(NCORES)))
    losses = np.concatenate([res.results[i]["loss"].reshape(-1)
                             for i in range(NCORES)])
    return np.float32(losses.mean(dtype=np.float64))



# revision 2
# speedup vs baseline: 2.0455x; 2.0455x over previous
"""Trainium2 Bass kernel for nn_ModelMultitaskBinary (MMoE multitask loss).

DEV LAYOUT: fp8 fast path lives in kernel_fp8.py; bf16 fallback in
kernel_bf16_backup.py. MUST BE INLINED into this file before shipping
(the harness stages kernel.py alone).

Data-parallel over batch B=512 across 8 cores (64 samples/core); params
replicated; no collectives — host averages the 8x64 per-sample losses.
"""
import os
import sys

for _p in ("/opt/trn_rl_repo", "/root/.axon_site/_ro/trn_rl_repo"):
    if os.path.isdir(_p) and _p not in sys.path:
        sys.path.insert(0, _p)

import numpy as np

from kernel_fp8 import (  # noqa: F401
    NCORES, BSH, build_nc_fp8, host_prep_fp8, KNOBS_FP8)
from concourse.bass_utils import run_bass_kernel_spmd

_CACHED = {}


def _zero_bias(inputs):
    return not any(
        np.asarray(inputs[k], np.float32).any()
        for k in ("fc1_b", "fc2_b", "expert_b1", "expert_b2",
                  "tower_b1", "tower_b2"))


def host_prep(inputs):
    zb = _zero_bias(inputs)
    if zb:
        return host_prep_fp8(inputs), True
    import kernel_bf16_backup as old
    return old.host_prep(inputs)


def get_nc(zero_bias=True, knobs=None):
    if zero_bias:
        key = ("fp8", tuple(sorted((knobs or KNOBS_FP8).items())))
        if key not in _CACHED:
            _CACHED[key] = build_nc_fp8(knobs)
        return _CACHED[key]
    import kernel_bf16_backup as old
    return old.get_nc(False)


def kernel(**inputs) -> np.ndarray:
    in_maps, zb = host_prep(inputs)
    nc = get_nc(zb)
    res = run_bass_kernel_spmd(nc, in_maps, list(range(NCORES)))
    losses = np.concatenate([res.results[i]["loss"].reshape(-1)
                             for i in range(NCORES)])
    return np.float32(losses.mean(dtype=np.float64))
